# revision 22
# baseline (speedup 1.0000x reference)
"""InterpretableMultiHeadAttention Trainium2 kernel.

B=8 batch elements -> 8 NeuronCores, data-parallel (one batch element per
core). All heads share weights in the reference module, so the computation
is a single-head attention; attn is broadcast 4x on the host.

Per-core dataflow (T=2048, D=256, DK=64):
  phase 0: load q,k,v [T,D] (chunked); PE-transpose to qT,kT,vT [D,T];
           qsT = Wq.T @ qT [DK,T], ksT likewise (fp32r matmuls);
           vs = v @ Wv [T,DK] (natural layout, fp32).
  main loop per 128-row tile n (A and B interleaved so ACT/DMA/PE overlap):
    pass A: scores[t-tile n] in 2 half-row chunks -> exp (accum_out partial
            rowsums) -> combine + reciprocal -> normalize -> DMA attn tile.
    pass B: scoresT[s-tile n] chunks (swap q/k) -> exp (unnormalized) ->
            head_rawT [DK,T] += vs_n.T @ expT accumulated in PSUM.
  phase C: out[t,:] = (head_rawT[:,t-tile]).T @ Wo, scaled by inv_rowsum
           per row (normalization commutes with the linear projections).
"""

import sys

sys.path.insert(0, "/opt/trn_rl_repo")

import numpy as np

import concourse.bacc as bacc
import concourse.bass as bass
import concourse.mybir as mybir
import concourse.tile as tile
from concourse.bass_utils import run_bass_kernel_spmd

B = 8
T = 2048
D = 256
DK = 64
NT = T // 128  # 16 tiles of 128 tokens
SCALE = 1.0 / np.sqrt(DK)

F32 = mybir.dt.float32
F32R = mybir.dt.float32r
EXP = mybir.ActivationFunctionType.Exp

_COMPILED = {}


def build_program(stage="all"):
    nc = bacc.Bacc("TRN2", target_bir_lowering=False, debug=False, num_devices=B)

    q_d = nc.dram_tensor("q", [T, D], F32, kind="ExternalInput").ap()
    k_d = nc.dram_tensor("k", [T, D], F32, kind="ExternalInput").ap()
    v_d = nc.dram_tensor("v", [T, D], F32, kind="ExternalInput").ap()
    wq_d = nc.dram_tensor("Wq", [D, DK], F32, kind="ExternalInput").ap()
    wk_d = nc.dram_tensor("Wk", [D, DK], F32, kind="ExternalInput").ap()
    wv_d = nc.dram_tensor("Wv", [D, DK], F32, kind="ExternalInput").ap()
    wo_d = nc.dram_tensor("Wo", [DK, D], F32, kind="ExternalInput").ap()
    id_d = nc.dram_tensor("ident", [128, 128], F32, kind="ExternalInput").ap()
    out_d = nc.dram_tensor("out", [T, D], F32, kind="ExternalOutput").ap()
    attn_d = nc.dram_tensor("attn", [T, T], F32, kind="ExternalOutput").ap()

    with tile.TileContext(nc) as tc:
        with (
            tc.tile_pool(name="persist", bufs=1) as pp,
            tc.tile_pool(name="stats", bufs=1) as sp,
            tc.tile_pool(name="psA", bufs=1, space="PSUM") as psA,
            tc.tile_pool(name="sbA", bufs=3) as sbA,
            tc.tile_pool(name="sbAn", bufs=2) as sbAn,
        ):
            ident = pp.tile([128, 128], F32)
            nc.sync.dma_start(ident[:], id_d[:])
            wq_s = pp.tile([128, 2, DK], F32, tag="wq_s")
            wk_s = pp.tile([128, 2, DK], F32, tag="wk_s")
            wv = pp.tile([128, 2, DK], F32, tag="wv")
            wo_s = pp.tile([DK, D], F32, tag="wo_s")
            nc.sync.dma_start(wq_s[:], wq_d.rearrange("(c p) d -> p c d", p=128))
            nc.sync.dma_start(wk_s[:], wk_d.rearrange("(c p) d -> p c d", p=128))
            nc.sync.dma_start(wv[:], wv_d.rearrange("(c p) d -> p c d", p=128))
            nc.sync.dma_start(wo_s[:], wo_d[:])
            wq = pp.tile([128, 2, DK], F32R, tag="wq")
            wk = pp.tile([128, 2, DK], F32R, tag="wk")
            wo = pp.tile([DK, D], F32R, tag="wo")
            nc.vector.tensor_copy(wq[:], wq_s[:])
            nc.vector.tensor_copy(wk[:], wk_s[:])
            nc.vector.tensor_copy(wo[:], wo_s[:])

            qsT = pp.tile([DK, T], F32R, tag="qsT")
            ksT = pp.tile([DK, T], F32R, tag="ksT")
            vs = pp.tile([128, NT, DK], F32R, tag="vs")
            rs2 = sp.tile([128, NT, 2], F32, tag="rs2")
            rowsum = sp.tile([128, NT], F32, tag="rowsum")
            inv = sp.tile([128, NT], F32, tag="inv")

            # ---- phase 0: load k,q; transpose+project; v deferred -------
            with (
                tc.tile_pool(name="nat", bufs=1) as natp,
                tc.tile_pool(name="xT", bufs=1) as xtp,
                tc.tile_pool(name="ps0", bufs=2, space="PSUM") as ps0,
                tc.tile_pool(name="ps0b", bufs=2, space="PSUM") as ps0b,
            ):
                nats = {}
                xTs = {}
                # chunked loads (4 tiles each) so transposes start early
                for name, dram in (("k", k_d), ("q", q_d), ("v", v_d)):
                    nat = natp.tile([128, NT, D], F32, tag=f"nat{name}")
                    drt = dram.rearrange("(n p) d -> p n d", p=128)
                    for ch in range(4):
                        nc.sync.dma_start(
                            nat[:, ch * 4 : (ch + 1) * 4, :],
                            drt[:, ch * 4 : (ch + 1) * 4, :],
                        )
                    nats[name] = nat
                    xdt = F32 if name == "v" else F32R
                    xTs[name] = xtp.tile(
                        [128, 2, T], xdt, tag=f"{name}T", name=f"{name}T"
                    )

                def transpose_tensor(name, w=None, dst=None, a_after=None):
                    nat, xT = nats[name], xTs[name]
                    for g in range(NT // 4):
                        tp = ps0.tile([128, 1024], F32, tag="tp")
                        for c in range(2):
                            for i in range(4):
                                n = g * 4 + i
                                nc.tensor.transpose(
                                    tp[:, c * 512 + i * 128 : c * 512 + (i + 1) * 128],
                                    nat[:, n, c * 128 : (c + 1) * 128],
                                    ident[:],
                                )
                        nc.vector.tensor_copy(
                            xT.rearrange("p c t -> p (c t)")[
                                :, g * 512 : g * 512 + 512
                            ],
                            tp[:, 0:512],
                        )
                        nc.vector.tensor_copy(
                            xT.rearrange("p c t -> p (c t)")[
                                :, T + g * 512 : T + g * 512 + 512
                            ],
                            tp[:, 512:1024],
                        )
                        if w is not None:
                            pj = ps0b.tile([DK, 512], F32, tag="proj")
                            for c in range(2):
                                nc.tensor.matmul(
                                    pj[:],
                                    w[:, c, :],
                                    xT[:, c, g * 512 : (g + 1) * 512],
                                    start=(c == 0),
                                    stop=(c == 1),
                                )
                            nc.vector.tensor_copy(
                                dst[:, g * 512 : (g + 1) * 512], pj[:]
                            )
                            if a_after is not None:
                                # qsT chunk g covers tiles 4g..4g+3
                                a_after(4 * g)

                def project_qk(w, xT, dst):
                    for j in range(T // 512):
                        pj = ps0b.tile([DK, 512], F32, tag="proj")
                        for c in range(2):
                            nc.tensor.matmul(
                                pj[:],
                                w[:, c, :],
                                xT[:, c, j * 512 : (j + 1) * 512],
                                start=(c == 0),
                                stop=(c == 1),
                            )
                        nc.vector.tensor_copy(dst[:, j * 512 : (j + 1) * 512], pj[:])

                def a_tile(n):
                    # pass A tile n: two half-row chunks
                    exs = []
                    for h in range(2):
                        scA = psA.tile([128, 1024], F32, tag="A")
                        for j in range(2):
                            lo = h * 1024 + j * 512
                            nc.tensor.matmul(
                                scA[:, j * 512 : (j + 1) * 512],
                                qsT[:, n * 128 : (n + 1) * 128],
                                ksT[:, lo : lo + 512],
                                start=True,
                                stop=True,
                            )
                        ex = sbA.tile([128, 1024], F32, tag="exp")
                        nc.scalar.activation(
                            ex[:], scA[:], EXP, scale=SCALE,
                            accum_out=rs2[:, n, h : h + 1],
                        )
                        exs.append(ex)
                    nc.vector.tensor_add(
                        rowsum[:, n : n + 1], rs2[:, n, 0:1], rs2[:, n, 1:2]
                    )
                    nc.vector.reciprocal(inv[:, n : n + 1], rowsum[:, n : n + 1])
                    at = sbAn.tile([128, T], F32, tag="attn")
                    for h in range(2):
                        nc.vector.tensor_scalar_mul(
                            at[:, h * 1024 : (h + 1) * 1024],
                            exs[h][:],
                            inv[:, n : n + 1],
                        )
                    nc.sync.dma_start(attn_d[n * 128 : (n + 1) * 128, :], at[:])

                transpose_tensor("k", wk, ksT)
                transpose_tensor("q", wq, qsT, a_after=a_tile)

                transpose_tensor("v")
                for n in range(NT):
                    pv = ps0b.tile([128, DK], F32, tag="proj", name="pv")
                    for c in range(2):
                        nc.tensor.matmul(
                            pv[:],
                            xTs["v"][:, c, n * 128 : (n + 1) * 128],
                            wv[:, c, :],
                            start=(c == 0),
                            stop=(c == 1),
                        )
                    nc.vector.tensor_copy(vs[:, n, :], pv[:])

            # ---- main loop: remaining A tiles + all B tiles interleaved --
            with (
                tc.tile_pool(name="psB", bufs=1, space="PSUM") as psB,
                tc.tile_pool(name="psH", bufs=1, space="PSUM") as psH,
                tc.tile_pool(name="sbB", bufs=3) as sbB,
            ):
                head = psH.tile([DK, T], F32, tag="head")

                def b_tile(n):
                    for h in range(2):
                        scB = psB.tile([128, 1024], F32, tag="Bp")
                        for j in range(2):
                            lo = h * 1024 + j * 512
                            nc.tensor.matmul(
                                scB[:, j * 512 : (j + 1) * 512],
                                ksT[:, n * 128 : (n + 1) * 128],
                                qsT[:, lo : lo + 512],
                                start=True,
                                stop=True,
                            )
                        eT = sbB.tile([128, 1024], F32R, tag="expT")
                        nc.scalar.activation(eT[:], scB[:], EXP, scale=SCALE)
                        for j in range(2):
                            lo = h * 1024 + j * 512
                            nc.tensor.matmul(
                                head[:, lo : lo + 512],
                                vs[:, n, :],
                                eT[:, j * 512 : (j + 1) * 512],
                                start=(n == 0),
                                stop=(n == NT - 1),
                            )

                rem = [n for n in range(NT) if n % 4 != 0]
                for i, n in enumerate(rem):
                    a_tile(n)
                    b_tile(i)
                for n in range(len(rem), NT):
                    b_tile(n)

                # ---- phase C: copy head out of PSUM (chunked) -------
                head_sb = pp.tile([DK, T], F32R, tag="head_sb")
                for gj in range(4):
                    nc.scalar.copy(
                        head_sb[:, gj * 512 : (gj + 1) * 512],
                        head[:, gj * 512 : (gj + 1) * 512],
                    )

            # ---- phase C: out projection + normalize (pipelined) -----
            with (
                tc.tile_pool(name="psO", bufs=6, space="PSUM") as psO,
                tc.tile_pool(name="sbO", bufs=1) as sbO,
            ):
                ot = sbO.tile([128, NT, D], F32, tag="ot")
                odr = out_d.rearrange("(n p) d -> p n d", p=128)
                for n in range(NT):
                    po = psO.tile([128, D], F32, tag="O")
                    nc.tensor.matmul(
                        po[:],
                        head_sb[:, n * 128 : (n + 1) * 128],
                        wo[:],
                        start=True,
                        stop=True,
                    )
                    if n % 2 == 0:
                        nc.vector.tensor_scalar_mul(
                            ot[:, n, :], po[:], inv[:, n : n + 1]
                        )
                    else:
                        nc.scalar.activation(
                            ot[:, n, :], po[:], mybir.ActivationFunctionType.Copy,
                            scale=inv[:, n : n + 1],
                        )
                    if n % 4 == 3:
                        nc.sync.dma_start(
                            odr[:, n - 3 : n + 1, :], ot[:, n - 3 : n + 1, :]
                        )

    nc.compile()
    return nc


def kernel(q, k, v, Wq, Wk, Wv, Wo):
    q = np.ascontiguousarray(np.asarray(q, dtype=np.float32))
    k = np.ascontiguousarray(np.asarray(k, dtype=np.float32))
    v = np.ascontiguousarray(np.asarray(v, dtype=np.float32))
    Wq = np.ascontiguousarray(np.asarray(Wq, dtype=np.float32))
    Wk = np.ascontiguousarray(np.asarray(Wk, dtype=np.float32))
    Wv = np.ascontiguousarray(np.asarray(Wv, dtype=np.float32))
    Wo = np.ascontiguousarray(np.asarray(Wo, dtype=np.float32))
    ident = np.eye(128, dtype=np.float32)

    if "nc" not in _COMPILED:
        _COMPILED["nc"] = build_program()
    nc = _COMPILED["nc"]

    in_maps = [
        {
            "q": q[b], "k": k[b], "v": v[b],
            "Wq": Wq, "Wk": Wk, "Wv": Wv, "Wo": Wo,
            "ident": ident,
        }
        for b in range(B)
    ]
    res = run_bass_kernel_spmd(nc, in_maps, core_ids=list(range(B)))
    outs = np.stack([res.results[b]["out"] for b in range(B)])
    attn = np.stack([res.results[b]["attn"] for b in range(B)])
    attn_stacked = np.broadcast_to(attn[None], (4, B, T, T))
    return outs, attn_stacked
